# revision 14
# baseline (speedup 1.0000x reference)
"""KDE2D Trainium2 Bass kernel.

Reference computation (per (b,t) pair, B=16, T=64, N=512, grid 128x128):
  standardize points (mean/std ddof=1 over N), then
  density[gx,gy] = norm * sum_n exp(-c*(xg[gx]-x_n)^2) * exp(-c*(yg[gy]-y_n)^2)
  with c = 1/(2 h^2), norm = 1/(2 pi h^2).

Kernel strategy (data-parallel over the 1024 (b,t) pairs, 128 per core):
  exp(-c(g - x)^2) = [e^{-c g^2 + K}] * exp(2c*x*g - c*x^2 - K)
  The second factor is built per (bt, n-chunk) tile [n=128 part, g=128 free]
  with ONE ScalarE activation op: Exp(scale_p * GXROW + bias_p), where
  scale_p = 2c*x_p and bias_p = -c*x_p^2 - K are per-partition operands.
  bf16 tiles feed a 4-chunk accumulating PE matmul (contract n=512) into
  PSUM; the rank-1 factor beta_gx*beta_gy (which also carries norm and
  e^{2K}) is applied by one DVE scalar_tensor_tensor op, then DMA out.
  K keeps bf16/psum values in range (max product exponent 2*c*25 - 2K).
"""

import math

import numpy as np

import concourse.bass as bass
import concourse.bacc as bacc
import concourse.mybir as mybir
from concourse import tile
from concourse.bass_utils import run_bass_kernel_spmd

B, T, N, GRID = 16, 64, 512, 128
NCORES = 8
BT_PER_CORE = (B * T) // NCORES  # 128
NCHUNK = N // 128  # 4

F32 = mybir.dt.float32
BF16 = mybir.dt.bfloat16

_CACHE = {}


def _build(bw: float):
    h = float(bw)
    c = 1.0 / (2.0 * h * h)
    norm = 1.0 / (2.0 * math.pi * h * h)
    gmax = 5.0
    K = c * gmax * gmax / 2.0  # per-side exponent shift

    nc = bacc.Bacc("TRN2", target_bir_lowering=False)
    a_ext = nc.declare_dram_parameter("a", [BT_PER_CORE, N, 2], F32, isOutput=False)
    gx_ext = nc.declare_dram_parameter("gxrow", [128, GRID], F32, isOutput=False)
    idt_ext = nc.declare_dram_parameter("idt", [128, 128], F32, isOutput=False)
    bx_ext = nc.declare_dram_parameter("betax", [128, 1], F32, isOutput=False)
    by_ext = nc.declare_dram_parameter("betay", [128, GRID], F32, isOutput=False)
    out_ext = nc.declare_dram_parameter(
        "out", [BT_PER_CORE, GRID, GRID], F32, isOutput=True
    )

    AT = mybir.ActivationFunctionType
    OP = mybir.AluOpType

    with tile.TileContext(nc) as tc:
        with (
            tc.tile_pool(name="const", bufs=1) as cpool,
            tc.tile_pool(name="stats", bufs=1) as spool,
            tc.tile_pool(name="work", bufs=3) as wpool,
            tc.tile_pool(name="exy", bufs=12) as epool,
            tc.tile_pool(name="psum", bufs=6, space="PSUM") as ppool,
            tc.tile_pool(name="psumT", bufs=2, space="PSUM") as tpool,
            tc.tile_pool(name="outp", bufs=6) as opool,
        ):
            gx_sb = cpool.tile([128, GRID], F32, tag="gx")
            idt_sb = cpool.tile([128, 128], F32, tag="idt")
            bx_sb = cpool.tile([128, 1], F32, tag="bx")
            by_sb = cpool.tile([128, GRID], F32, tag="by")
            nc.sync.dma_start(gx_sb[:], gx_ext[:])
            nc.sync.dma_start(idt_sb[:], idt_ext[:])
            nc.sync.dma_start(bx_sb[:], bx_ext[:])
            nc.sync.dma_start(by_sb[:], by_ext[:])

            # ---- load points contiguously: [bt(128 part), n, ch] ----
            a_all = spool.tile([128, N, 2], F32, tag="a")
            nc.sync.dma_start(a_all[:], a_ext[:])
            x_sb = a_all[:, :, 0]
            y_sb = a_all[:, :, 1]

            # ---- per-bt stats and derived scale/bias arrays (layout [bt, n]) ----
            # sx = 2c * (x-mean)*invsd ; biasx = -c*((x-mean)*invsd)^2 - K
            derived = {}
            for ch, src in (("x", x_sb), ("y", y_sb)):
                s1 = spool.tile([128, 1], F32, tag=f"s1{ch}")
                s2 = spool.tile([128, 1], F32, tag=f"s2{ch}")
                sq = wpool.tile([128, N], F32, tag="sq")
                nc.vector.tensor_reduce(s1[:], src, mybir.AxisListType.X, OP.add)
                nc.vector.tensor_tensor(sq[:], src, src, OP.mult)
                nc.vector.tensor_reduce(s2[:], sq[:], mybir.AxisListType.X, OP.add)
                mean = spool.tile([128, 1], F32, tag=f"mean{ch}")
                nc.vector.tensor_scalar_mul(mean[:], s1[:], 1.0 / N)
                m2 = spool.tile([128, 1], F32, tag=f"m2{ch}")
                nc.vector.tensor_tensor(m2[:], mean[:], mean[:], OP.mult)
                var = spool.tile([128, 1], F32, tag=f"var{ch}")
                # var = (s2 - N*m2) / (N-1)
                nc.vector.scalar_tensor_tensor(
                    var[:], m2[:], -float(N), s2[:], OP.mult, OP.add
                )
                nc.vector.tensor_scalar_mul(var[:], var[:], 1.0 / (N - 1))
                sd = spool.tile([128, 1], F32, tag=f"sd{ch}")
                nc.scalar.activation(sd[:], var[:], AT.Sqrt)
                invsd = spool.tile([128, 1], F32, tag=f"invsd{ch}")
                nc.vector.reciprocal(invsd[:], sd[:])

                # xt = (x - mean) * invsd  (two tensor_scalar ops)
                xt = wpool.tile([128, N], F32, tag=f"xt{ch}")
                nc.vector.tensor_scalar(
                    xt[:], src, mean[:, 0:1], None, OP.subtract
                )
                nc.vector.tensor_scalar(
                    xt[:], xt[:], invsd[:, 0:1], None, OP.mult
                )
                # scale array: 2c * xt
                sc = wpool.tile([128, N], F32, tag=f"sc{ch}")
                nc.vector.tensor_scalar_mul(sc[:], xt[:], 2.0 * c)
                # bias array: -c*xt^2 - K
                bi = wpool.tile([128, N], F32, tag=f"bi{ch}")
                nc.vector.tensor_tensor(bi[:], xt[:], xt[:], OP.mult)
                nc.vector.tensor_scalar(bi[:], bi[:], -c, -K, OP.mult, OP.add)
                derived[ch] = (sc, bi)

            # ---- transpose derived arrays to [n(part), bt] via PE ----
            # Matmult instructions only tolerate ONE sync wait in walrus
            # codegen, so absorb the idt/gx DMA ticks into PE/ACT clocks
            # with dummy ops before the real transposes run.
            dummy_pt = tpool.tile([128, 128], F32, tag="pt")
            nc.tensor.transpose(dummy_pt[:], idt_sb[:], idt_sb[:])
            gx_probe = spool.tile([128, 1], F32, tag="gxprobe")
            nc.scalar.activation(gx_probe[:], gx_sb[:, 0:1], AT.Copy)
            # arrT[cc][:, bt] columns feed activation scale/bias operands.
            trans = {}
            for name, arr in (
                ("scx", derived["x"][0]),
                ("bix", derived["x"][1]),
                ("scy", derived["y"][0]),
                ("biy", derived["y"][1]),
            ):
                tiles = []
                for cc in range(NCHUNK):
                    pt = tpool.tile([128, 128], F32, tag="pt")
                    nc.tensor.transpose(
                        pt[:], arr[:, cc * 128 : (cc + 1) * 128], idt_sb[:]
                    )
                    st = cpool.tile([128, 128], F32, tag=f"T{name}{cc}")
                    nc.vector.tensor_copy(st[:], pt[:])
                    tiles.append(st)
                trans[name] = tiles

            # ---- main loop: one (bt) per iteration ----
            for bt in range(BT_PER_CORE):
                ps = ppool.tile([128, GRID], F32, tag="ps")
                for cc in range(NCHUNK):
                    ex = epool.tile([128, GRID], BF16, tag="ex")
                    ey = epool.tile([128, GRID], BF16, tag="ey")
                    nc.scalar.activation(
                        ex[:], gx_sb[:], AT.Exp,
                        bias=trans["bix"][cc][:, bt : bt + 1],
                        scale=trans["scx"][cc][:, bt : bt + 1],
                    )
                    nc.scalar.activation(
                        ey[:], gx_sb[:], AT.Exp,
                        bias=trans["biy"][cc][:, bt : bt + 1],
                        scale=trans["scy"][cc][:, bt : bt + 1],
                    )
                    nc.tensor.matmul(
                        ps[:], ex[:], ey[:],
                        start=(cc == 0), stop=(cc == NCHUNK - 1),
                    )
                ob = opool.tile([128, GRID], F32, tag="ob")
                # out = (psum * betax_p) * betay_row  (one DVE op)
                nc.vector.scalar_tensor_tensor(
                    ob[:], ps[:], bx_sb[:, 0:1], by_sb[:], OP.mult, OP.mult
                )
                nc.sync.dma_start(out_ext[bt], ob[:])

    if not nc.is_finalized():
        nc.finalize()
    return nc


def _consts(bw: float):
    h = float(bw)
    c = 1.0 / (2.0 * h * h)
    norm = 1.0 / (2.0 * math.pi * h * h)
    gmax = 5.0
    K = c * gmax * gmax / 2.0
    xg = np.linspace(-5.0, 5.0, GRID, dtype=np.float64)
    gxrow = np.broadcast_to(xg.astype(np.float32), (128, GRID)).copy()
    idt = np.eye(128, dtype=np.float32)
    betax = np.exp(K - c * xg * xg).astype(np.float32).reshape(GRID, 1)
    betay = (norm * np.exp(K - c * xg * xg)).astype(np.float32)
    betay = np.broadcast_to(betay, (128, GRID)).copy()
    return gxrow, idt, betax, betay


def kernel(A: np.ndarray, bandwidth: np.ndarray) -> np.ndarray:
    A = np.asarray(A, dtype=np.float32)
    bw = float(np.asarray(bandwidth))
    key = round(bw, 9)
    if key not in _CACHE:
        _CACHE[key] = _build(bw)
    nc = _CACHE[key]

    gxrow, idt, betax, betay = _consts(bw)
    a_flat = A.reshape(B * T, N, 2)
    in_maps = []
    for i in range(NCORES):
        in_maps.append(
            {
                "a": np.ascontiguousarray(
                    a_flat[i * BT_PER_CORE : (i + 1) * BT_PER_CORE]
                ),
                "gxrow": gxrow,
                "idt": idt,
                "betax": betax,
                "betay": betay,
            }
        )
    res = run_bass_kernel_spmd(nc, in_maps, core_ids=list(range(NCORES)))
    outs = [res.results[i]["out"] for i in range(NCORES)]
    return np.concatenate(outs, axis=0).reshape(B, T, GRID, GRID)


if __name__ == "__main__":
    A = np.random.randn(B, T, N, 2).astype(np.float32)
    out = kernel(A, np.float32(0.5))
    print(out.shape, out.dtype, float(out.max()))
